# revision 26
# baseline (speedup 1.0000x reference)
"""Trainium2 Bass kernel for nn_ExpansionContrastModule (v2).

Strategy: pure data parallel over 8 cores; each core processes half of one
batch image (128 of 256 rows) with a 3-row halo.

v2 redesign vs v1 baseline:
- cen is loaded ONCE as bf16 and stays resident in SBUF; the final gating
  multiply reads it from SBUF (v1 re-read cen as f32 from HBM: -17MB/core).
- output is written bf16 and converted to f32 on the host (-8MB/core).
- contrast stage uses the difference-product identity
  (x - x_d)(x - x_{-d}) = -D_d(p) * D_d(p - d)  with D_d = x - shift_d(x),
  computed once per direction on an extended domain: 2 DVE ops per
  direction instead of 3 (min/max/sum trees run on negated products with
  swapped/negated weights).
- max(t1,t3) and (t1+t3) are never materialized:
  y = v0*mn + v1m*(t1+t3) + v2*(t1+t3-mn) folds into matmul weights
  (v0-v2)*W for mn and (v1m+v2)*W for t1,t3.
- ~40% of elementwise work + the final multiply run on the Pool engine
  (gpsimd); the final multiply reads the gate straight from PSUM.
- mas 3x3 conv input built with 3 gather-DMAs (overlapping APs) from a
  host-padded [130,258] tensor instead of 18 strided DMAs.

Per-core layout: channels of the reduced tensor x (16) are packed 8 groups
to the 128 SBUF partitions (partition 16g+c = row-group g, channel c).
Two sequential 64-row macro-halves per core.
"""
import dataclasses
import sys

import ml_dtypes
import numpy as np

sys.path.insert(0, "/opt/trn_rl_repo")

import concourse.bass as bass
import concourse.bacc as bacc
import concourse.mybir as mybir
from concourse.tile import TileContext
from concourse.bass_utils import run_bass_kernel_spmd

F32 = mybir.dt.float32
BF = mybir.dt.bfloat16
AF = mybir.ActivationFunctionType
ALU = mybir.AluOpType

N_CORES = 8
C = 128        # input channels
CR = 16        # reduced channels
H = W = 256
CH = 128       # rows per core (half an image)
MH = 2         # macro-halves per core
HB = 64        # rows per macro-half
G = 8          # row-groups per macro-half
GR = 8         # rows per group
XR = GR + 6    # x tile rows (3-row halo each side)
XP = 4         # x tile left/right col pad
XW = W + 2 * XP  # 264
QF = GR * W    # 2048 free elems per macro-half slab
J0 = 3         # x tile row index of the first slab row

BN_EPS = 1e-5

# cbf column blocks
CB_WIN = 0          # [C, 8*C]      w_in block-diag
CB_BC = 1024        # [C, 3*C]      bc conv weights for (mn, t1, t3)
CB_WOUT = 1408      # [C, 8]        w_out block
CB_MAS = 1416       # [72, 8]       mas 3x3 kernel
CB_BCAST = 1424     # [40, 8*C]     gate broadcast (ct_g + ge2_{32+g} per group)
NCB = 2448

# cf32 scalar columns
S_BIN, S_BTOP, S_BBOT, S_BNS, S_BNB = 0, 1, 2, 3, 4
S_NW0, S_NW1_4, S_NW2 = 5, 6, 7
S_BT0, S_BT1 = 8, 9
S_BOUT, S_MB1, S_MW2, S_MB2 = 12, 13, 14, 15
NCF = 16

_CACHE = {}


def _fl(t):
    """Flatten the two free dims of a [P, a, b] tile AP into [P, a*b]."""
    return t[:, :, :].rearrange("p a b -> p (a b)")


def build_nc(loop_reps=0):
    nc = bacc.Bacc("TRN2", target_bir_lowering=False, debug=False,
                   num_devices=N_CORES)
    cen_d = nc.dram_tensor("cen_bf", [C, CH + 6, W], BF, kind="ExternalInput")
    mas_d = nc.dram_tensor("mas", [CH + 2, W + 2], BF, kind="ExternalInput")
    cbf_d = nc.dram_tensor("cbf", [C, NCB], BF, kind="ExternalInput")
    cf32_d = nc.dram_tensor("cf32", [C, NCF], F32, kind="ExternalInput")
    out_d = nc.dram_tensor("out", [C, CH, W], BF, kind="ExternalOutput")

    with TileContext(nc) as tc:
        import contextlib
        _stk = contextlib.ExitStack()
        with _stk:
            ep = _stk.enter_context
            cpool = ep(tc.tile_pool(name="const", bufs=1))
            cenpool = ep(tc.tile_pool(name="cen", bufs=1))
            xpool = ep(tc.tile_pool(name="x", bufs=2))
            xopool = ep(tc.tile_pool(name="xo", bufs=2))
            dpool = ep(tc.tile_pool(name="d", bufs=3))
            dopool = ep(tc.tile_pool(name="do", bufs=1))
            upool = ep(tc.tile_pool(name="u", bufs=2))
            trpool = ep(tc.tile_pool(name="tr", bufs=5))
            tmppool = ep(tc.tile_pool(name="tm", bufs=2))
            tpool = ep(tc.tile_pool(name="t", bufs=3))
            zpool = ep(tc.tile_pool(name="z", bufs=1))
            gpool = ep(tc.tile_pool(name="g", bufs=1))
            mpool = ep(tc.tile_pool(name="m", bufs=1))
            m9pool = ep(tc.tile_pool(name="m9", bufs=1))
            opool = ep(tc.tile_pool(name="o", bufs=2))
            pxpool = ep(tc.tile_pool(name="px", bufs=2, space="PSUM"))
            smpool = ep(tc.tile_pool(name="sm", bufs=2, space="PSUM"))
            pgpool = ep(tc.tile_pool(name="pg", bufs=2, space="PSUM"))

            # ---- constants ----
            cbf_sb = cpool.tile([C, NCB], BF, tag="c_bf")
            cf32_sb = cpool.tile([C, NCF], F32, tag="c_f32")
            nc.sync.dma_start(out=cbf_sb[:], in_=cbf_d[:])
            nc.sync.dma_start(out=cf32_sb[:], in_=cf32_d[:])

            def sc(col, p=C):
                return cf32_sb[0:p, col:col + 1]

            def stage_A(mh, cen_fl, x, with_mas9=True):
                """mas9 gather (opt), x conv, halos, x_odd. PE/Act/SP work."""
                base = mh * HB
                mas9 = None
                if with_mas9:
                    mas9 = emit_mas9(mh)
                for c2 in range(4):
                    px = pxpool.tile([C, 512], F32, tag="px")
                    for g in range(G):
                        r0 = (base + 8 * g + J0 + 2 * c2) * W
                        nc.tensor.matmul(
                            px[:], cbf_sb[:, CB_WIN + g * C:CB_WIN + (g + 1) * C],
                            cen_fl[:, r0:r0 + 512],
                            start=(g == 0), stop=(g == G - 1))
                    nc.scalar.activation(
                        x[:, J0 + 2 * c2:J0 + 2 * c2 + 2, XP:XP + W],
                        px[:].rearrange("p (a b) -> p a b", a=2),
                        AF.Identity, bias=sc(S_BIN), scale=1.0)
                # edge rows: top (group 0 only), bottom (group 7 only)
                bt_c = S_BTOP if mh == 0 else S_BIN
                bb_c = S_BBOT if mh == MH - 1 else S_BIN
                for j in range(3):
                    pe = pxpool.tile([C, 512], F32, tag="px")
                    nc.tensor.matmul(pe[:, 0:W], cbf_sb[:, CB_WIN:CB_WIN + C],
                                     cen_fl[:, (base + j) * W:(base + j + 1) * W],
                                     start=True, stop=True)
                    nc.scalar.activation(
                        x[0:CR, j, XP:XP + W], pe[0:CR, 0:W], AF.Identity,
                        bias=sc(bt_c, CR), scale=1.0)
                    pe2 = pxpool.tile([C, 512], F32, tag="px")
                    nc.tensor.matmul(
                        pe2[:, 0:W], cbf_sb[:, CB_WIN + 7 * C:CB_WIN + 8 * C],
                        cen_fl[:, (base + 56 + 11 + j) * W:(base + 56 + 12 + j) * W],
                        start=True, stop=True)
                    # start partition must be a multiple of 32; rows 96:112 get
                    # junk and are re-written by the halo DMA below.
                    nc.scalar.activation(
                        x[96:C, 11 + j, XP:XP + W], pe2[96:C, 0:W],
                        AF.Identity, bias=cf32_sb[96:C, bb_c:bb_c + 1], scale=1.0)
                # interior halos between groups via partition-shifted SBUF DMA
                nc.sync.dma_start(out=x[CR:C, 0:3, XP:XP + W],
                                  in_=x[0:C - CR, GR:GR + 3, XP:XP + W])
                nc.sync.dma_start(out=x[0:C - CR, GR + 3:GR + 6, XP:XP + W],
                                  in_=x[CR:C, 3:6, XP:XP + W])
                # x_odd[c] = x[c+1]: one flat shifted copy (Act)
                x_odd = xopool.tile([C, XR, XW], BF, tag="xo")
                nc.scalar.copy(_fl(x_odd)[:, 0:XR * XW - 1], _fl(x)[:, 1:XR * XW])
                return {'x': x, 'xo': x_odd, 'mas9': mas9}

            def emit_mas9(mh):
                """3 overlapping-AP gather DMAs building the 9-shift layout."""
                base = mh * HB
                mas9 = m9pool.tile([72, GR, W], BF, tag="m9")
                msrc = mas_d[:]
                for dy in range(3):
                    apd = dataclasses.replace(
                        msrc, offset=(base + dy) * (W + 2),
                        ap=[[1, 3], [GR * (W + 2), G], [W + 2, GR], [1, W]])
                    nc.sync.dma_start(out=mas9[24 * dy:24 * (dy + 1)], in_=apd)
                return mas9

            def stage_B(st):
                """mas conv part 1: PE matmuls + Act silu (exact, via LUT)."""
                m9f = _fl(st['mas9'])
                msl = mpool.tile([G, QF], BF, tag="msl")
                for c2 in range(4):
                    pm = smpool.tile([C, 512], F32, tag="sm")
                    nc.tensor.matmul(pm[0:G, :], cbf_sb[0:72, CB_MAS:CB_MAS + 8],
                                     m9f[:, 512 * c2:512 * (c2 + 1)],
                                     start=True, stop=True)
                    nc.scalar.activation(msl[:, 512 * c2:512 * (c2 + 1)],
                                         pm[0:G, :], AF.Silu,
                                         bias=sc(S_MB1, G), scale=1.0)
                st['msl'] = msl

            def stage_C(st):
                """Contrast stage: DVE-heavy with Pool offload."""
                x, x_odd = st['x'], st['xo']
                t_tiles = []
                for s in (1, 3):
                    nr = GR + s
                    wd = W + s + 1  # even extended width for diag/col dirs
                    Db = dpool.tile([C, 11, XW], BF, tag="d")
                    nc.vector.tensor_tensor(
                        Db[:, 0:nr, 4:4 + W], x[:, J0:J0 + nr, XP:XP + W],
                        x[:, J0 - s:J0 - s + nr, XP:XP + W], ALU.subtract)
                    Da = dpool.tile([C, 11, XW], BF, tag="d")
                    nc.vector.tensor_tensor(
                        Da[:, 0:nr, 4:4 + wd], x[:, J0:J0 + nr, XP:XP + wd],
                        x_odd[:, J0 - s:J0 - s + nr, XP - s - 1:XP - s - 1 + wd],
                        ALU.subtract)
                    Doa = dopool.tile([C, GR, W], BF, tag="do")
                    nc.scalar.copy(
                        Doa[:], Da[:, s:s + GR, s + 4:s + 4 + W])
                    ub = upool.tile([C, GR, W], BF, tag="u")
                    nc.vector.tensor_tensor(
                        ub[:], Db[:, 0:GR, 4:4 + W], Db[:, s:s + GR, 4:4 + W],
                        ALU.mult)
                    ua = upool.tile([C, GR, W], BF, tag="u")
                    nc.vector.tensor_tensor(
                        ua[:], Da[:, 0:GR, 4:4 + W], Doa[:], ALU.mult)
                    m01 = trpool.tile([C, QF], BF, tag="tr")
                    nc.vector.tensor_tensor(m01[:], _fl(ua), _fl(ub), ALU.min)
                    s01 = trpool.tile([C, QF], BF, tag="tr")
                    nc.gpsimd.tensor_tensor(s01[:], _fl(ua), _fl(ub), ALU.add)
                    M01 = trpool.tile([C, QF], BF, tag="tr")
                    nc.vector.tensor_tensor(M01[:], _fl(ua), _fl(ub), ALU.max)

                    Dc = dpool.tile([C, 11, XW], BF, tag="d")
                    nc.vector.tensor_tensor(
                        Dc[:, 0:nr, 4:4 + wd],
                        x_odd[:, J0:J0 + nr, XP - s - 1:XP - s - 1 + wd],
                        x[:, J0 - s:J0 - s + nr, XP:XP + wd], ALU.subtract)
                    Doc = dopool.tile([C, GR, W], BF, tag="do")
                    nc.scalar.copy(
                        Doc[:], Dc[:, 0:GR, s + 4:s + 4 + W])
                    uc = upool.tile([C, GR, W], BF, tag="u")
                    nc.vector.tensor_tensor(
                        uc[:], Doc[:], Dc[:, s:s + GR, 4:4 + W], ALU.mult)
                    Dd = dpool.tile([C, 11, XW], BF, tag="d")
                    nc.vector.tensor_tensor(
                        Dd[:, 0:GR, 4:4 + wd], x[:, J0:J0 + GR, XP:XP + wd],
                        x_odd[:, J0:J0 + GR, XP - s - 1:XP - s - 1 + wd],
                        ALU.subtract)
                    Dod = dopool.tile([C, GR, W], BF, tag="do")
                    nc.scalar.copy(
                        Dod[:], Dd[:, 0:GR, s + 4:s + 4 + W])
                    ud = upool.tile([C, GR, W], BF, tag="u")
                    nc.vector.tensor_tensor(
                        ud[:], Dd[:, 0:GR, 4:4 + W], Dod[:], ALU.mult)

                    m23 = trpool.tile([C, QF], BF, tag="tr")
                    nc.vector.tensor_tensor(m23[:], _fl(uc), _fl(ud), ALU.min)
                    s23 = trpool.tile([C, QF], BF, tag="tr")
                    nc.gpsimd.tensor_tensor(s23[:], _fl(uc), _fl(ud), ALU.add)
                    min4 = tmppool.tile([C, QF], BF, tag="tmp")
                    nc.vector.tensor_tensor(min4[:], m01[:], m23[:], ALU.min)
                    M23 = trpool.tile([C, QF], BF, tag="tr")
                    nc.vector.tensor_tensor(M23[:], _fl(uc), _fl(ud), ALU.max)
                    max4 = tmppool.tile([C, QF], BF, tag="tmp")
                    nc.vector.tensor_tensor(max4[:], M01[:], M23[:], ALU.max)
                    sum4 = tmppool.tile([C, QF], BF, tag="tmp")
                    nc.gpsimd.tensor_tensor(sum4[:], s01[:], s23[:], ALU.add)

                    # t_s = -(w0*max4 + w1m*sum4 + w2*min4)   (u = -o)
                    qa = trpool.tile([C, QF], BF, tag="tr")
                    nc.vector.tensor_scalar_mul(qa[:], max4[:], sc(S_NW0))
                    qb = trpool.tile([C, QF], BF, tag="tr")
                    nc.vector.tensor_scalar_mul(qb[:], min4[:], sc(S_NW2))
                    q1 = trpool.tile([C, QF], BF, tag="tr")
                    nc.vector.tensor_scalar_mul(q1[:], sum4[:], sc(S_NW1_4))
                    qu = trpool.tile([C, QF], BF, tag="tr")
                    nc.gpsimd.tensor_tensor(qu[:], qa[:], qb[:], ALU.add)
                    t_s = tpool.tile([C, QF], BF, tag="ts")
                    nc.gpsimd.tensor_tensor(t_s[:], qu[:], q1[:], ALU.add)
                    t_tiles.append(t_s)

                t1, t3 = t_tiles
                mn = tpool.tile([C, QF], BF, tag="ts")
                nc.vector.tensor_tensor(mn[:], t1[:], t3[:], ALU.min)
                st['t1'], st['t3'], st['mn'] = t1, t3, mn

            def stage_D(mh, st, cen_sb, gate16):
                """bc+bn+silu, w_out, gate combine (Pool), bcast+final, out."""
                base = mh * HB
                t1, t3, mn = st['t1'], st['t3'], st['mn']
                zq = zpool.tile([C, QF], BF, tag="zq")
                for c2 in range(4):
                    cc = slice(512 * c2, 512 * (c2 + 1))
                    pb = pxpool.tile([C, 512], F32, tag="px")
                    nc.tensor.matmul(pb[:], cbf_sb[:, CB_BC:CB_BC + C],
                                     mn[:, cc], start=True, stop=False)
                    nc.tensor.matmul(pb[:], cbf_sb[:, CB_BC + C:CB_BC + 2 * C],
                                     t1[:, cc], start=False, stop=False)
                    nc.tensor.matmul(pb[:], cbf_sb[:, CB_BC + 2 * C:CB_BC + 3 * C],
                                     t3[:, cc], start=False, stop=True)
                    nc.scalar.activation(zq[:, cc], pb[:], AF.Silu,
                                         bias=sc(S_BNB), scale=sc(S_BNS))

                om_q = gpool.tile([G, QF], BF, tag="om")
                for c2 in range(4):
                    pu = smpool.tile([C, 512], F32, tag="sm")
                    nc.tensor.matmul(pu[0:G, :], cbf_sb[:, CB_WOUT:CB_WOUT + 8],
                                     zq[:, 512 * c2:512 * (c2 + 1)],
                                     start=True, stop=True)
                    nc.scalar.activation(om_q[:, 512 * c2:512 * (c2 + 1)],
                                         pu[0:G, :], AF.Tanh,
                                         bias=sc(S_BOUT, G), scale=0.5)

                # h_mm at base partition 0 (HW: SBUF STT operands must share
                # base partition), then copied to gate40 rows 32:40 (weight
                # s1p); row 8 is constant 1.0 (weight s3p); rows 0:8 =
                # (h_mm+s0p/s2p)*h_om (weight s2p).
                mm_q = mpool.tile([G, QF], BF, tag="mm")
                nc.scalar.activation(mm_q[:], st['msl'][:], AF.Tanh,
                                     bias=sc(S_MB2, G), scale=sc(S_MW2, G))
                nc.vector.tensor_copy(gate16[32:40, :], mm_q[:])
                bt = mpool.tile([G, QF], BF, tag="bt")
                nc.scalar.activation(bt[:], mm_q[:], AF.Identity,
                                     bias=sc(S_BT0, G), scale=sc(S_BT1, G))
                nc.vector.tensor_tensor(gate16[0:G, :], om_q[:], bt[:], ALU.mult)

                # final multiply: GPSIMD cannot read PSUM on HW, so half the
                # groups multiply on DVE straight from PSUM (fp32, 1x) and
                # half go Act-evac (bf16) -> Pool multiply.
                for g in range(G):
                    out_t = opool.tile([C, GR, W], BF, tag="o")
                    lh = cbf_sb[0:40, CB_BCAST + g * C:CB_BCAST + (g + 1) * C]
                    for h in range(2):
                        pg = pgpool.tile([C, 1024], F32, tag="pg")
                        for c4 in range(2):
                            c2 = 2 * h + c4
                            nc.tensor.matmul(
                                pg[:, 512 * c4:512 * (c4 + 1)], lh,
                                gate16[:, 512 * c2:512 * (c2 + 1)],
                                start=True, stop=True)
                        r0 = base + 8 * g + 4 * h
                        cs = cen_sb[:, 3 + r0:3 + r0 + 4, :]
                        ot = out_t[:, 4 * h:4 * h + 4, :]
                        pgv = pg[:].rearrange("p (a b) -> p a b", a=4)
                        if g % 2 == 0:
                            nc.vector.tensor_tensor(ot, cs, pgv, ALU.mult)
                        else:
                            gs = trpool.tile([C, QF], BF, tag="tr")
                            gsv = gs[:, 0:1024].rearrange(
                                "p (a b) -> p a b", a=4)
                            nc.scalar.activation(gsv, pgv, AF.Identity,
                                                 bias=0.0, scale=1.0)
                            nc.vector.tensor_tensor(ot, cs, gsv, ALU.mult)
                    eng = nc.scalar if g % 4 == 3 else nc.sync
                    eng.dma_start(
                        out=out_d[:, base + 8 * g:base + 8 * g + GR, :],
                        in_=out_t[:])

            def emit_all():
                # PE p-state warmup: dummy matmuls so the clock ramps while
                # cen streams in (results never read)
                pw = pxpool.tile([C, 512], F32, tag="px")
                for i in range(10):
                    nc.tensor.matmul(pw[:], cbf_sb[:, 0:C],
                                     cbf_sb[:, 0:512], start=(i == 0),
                                     stop=(i == 9))
                cen_sb = cenpool.tile([C, CH + 6, W], BF, tag="cen")
                nrows = [18, 18, 17, 17, 32, 32]
                r = 0
                for i, n in enumerate(nrows):
                    nc.sync.dma_start(out=cen_sb[:, r:r + n, :],
                                      in_=cen_d[:, r:r + n, :])
                    r += n
                cen_fl = _fl(cen_sb)
                gate16 = gpool.tile([40, QF], BF, tag="g16")
                nc.gpsimd.memset(gate16[0:32, :], 1.0)
                # x ring buffers allocated up front so their constant zero
                # col-pads are set once, off the critical path
                xs = []
                for _ in range(MH):
                    xt = xpool.tile([C, XR, XW], BF, tag="x")
                    nc.gpsimd.memset(xt[:, :, 0:XP], 0.0)
                    nc.gpsimd.memset(xt[:, :, W + XP:XW], 0.0)
                    xs.append(xt)
                # software-pipelined over the two macro-halves
                st0 = stage_A(0, cen_fl, xs[0])
                stage_B(st0)
                mas9_1 = emit_mas9(1)
                stage_C(st0)
                st1 = stage_A(1, cen_fl, xs[1], with_mas9=False)
                st1['mas9'] = mas9_1
                stage_D(0, st0, cen_sb, gate16)
                stage_B(st1)
                stage_C(st1)
                stage_D(1, st1, cen_sb, gate16)

            rep_ctx = (tc.For_i(0, loop_reps, 1) if loop_reps
                       else contextlib.nullcontext())
            with rep_ctx:
                emit_all()
    nc.compile()
    return nc


def _softmax(v):
    e = np.exp(v - v.max())
    return e / e.sum()


def _prep_consts(inp):
    w = _softmax(inp['scale1'])
    v = _softmax(inp['scale2'])
    s3 = _softmax(inp['scale3'])
    inv = inp['bn_gamma'] / np.sqrt(inp['bn_var'] + BN_EPS)
    bnb = inp['bn_beta'] - inp['bn_mean'] * inv

    w_in_blk = np.zeros((C, G, C), np.float32)
    for g in range(G):
        w_in_blk[:, g, CR * g:CR * (g + 1)] = inp['w_in'].T
    bc_blk = np.kron(np.eye(G), inp['bc_w'].T)
    A, B = v[0] - v[2], v[1] / 2.0 + v[2]
    wout_lhsT = np.kron(np.eye(G), inp['w_out'][0][:, None]).astype(np.float32)
    k_flat = inp['mas_w1'][0, 0].reshape(9)
    mas_lhsT = np.kron(k_flat[:, None], np.eye(G)).astype(np.float32)
    # gate in tanh basis: om = (1+h_om)/2, mm = (1+h_mm)/2 with
    # h = tanh(pre/2); gate = s3p + s0p*h_om + s1p*h_mm + s2p*h_om*h_mm
    s0p = s3[0] / 2 + s3[2] / 4
    s1p = s3[1] / 2 + s3[2] / 4
    s2p = s3[2] / 4
    s3p = s3[3] + s3[0] / 2 + s3[1] / 2 + s3[2] / 4
    bcast = np.zeros((40, G, C), np.float32)
    for g in range(G):
        bcast[g, g, :] = 1.0
        bcast[8, g, :] = s3p
        bcast[32 + g, g, :] = s1p

    cbf = np.zeros((C, NCB), np.float32)
    cbf[:, CB_WIN:CB_WIN + G * C] = w_in_blk.reshape(C, G * C)
    cbf[:, CB_BC:CB_BC + C] = A * bc_blk
    cbf[:, CB_BC + C:CB_BC + 2 * C] = B * bc_blk
    cbf[:, CB_BC + 2 * C:CB_BC + 3 * C] = B * bc_blk
    cbf[:, CB_WOUT:CB_WOUT + 8] = wout_lhsT
    cbf[0:72, CB_MAS:CB_MAS + 8] = mas_lhsT
    cbf[0:40, CB_BCAST:CB_BCAST + G * C] = bcast.reshape(40, G * C)

    b_in_t = np.tile(inp['b_in'], G)[:, None].astype(np.float32)
    cf32 = np.zeros((C, NCF), np.float32)
    cf32[:, S_BIN:S_BIN + 1] = b_in_t
    cf32[:, S_BNS:S_BNS + 1] = np.tile(inv, G)[:, None]
    cf32[:, S_BNB:S_BNB + 1] = np.tile(bnb, G)[:, None]
    scal = [(S_NW0, -w[0]), (S_NW1_4, -w[1] / 4.0), (S_NW2, -w[2]),
            (S_BT0, s0p), (S_BT1, s2p),
            (S_BOUT, inp['b_out'][0] / 2.0), (S_MB1, inp['mas_b1'][0]),
            (S_MW2, inp['mas_w2'][0, 0] / 2.0),
            (S_MB2, inp['mas_b2'][0] / 2.0)]
    for col, val in scal:
        cf32[:, col] = val
    return {'cbf': cbf.astype(ml_dtypes.bfloat16), 'cf32': cf32,
            'b_in_t': b_in_t}


def make_in_maps(inp, consts):
    in_maps = []
    for core in range(N_CORES):
        b, hf = core // 2, core % 2
        r0 = CH * hf
        cen_pad = np.pad(inp['cen'][b], ((0, 0), (3, 3), (0, 0)))
        cen_core = np.ascontiguousarray(cen_pad[:, r0:r0 + CH + 6, :])
        mas_pad = np.pad(inp['mas'][b, 0], ((1, 1), (1, 1)))
        mas_core = np.ascontiguousarray(mas_pad[r0:r0 + CH + 2, :])
        cf32 = consts['cf32'].copy()
        if hf != 0:
            cf32[:, S_BTOP:S_BTOP + 1] = consts['b_in_t']
        if hf != 1:
            cf32[:, S_BBOT:S_BBOT + 1] = consts['b_in_t']
        in_maps.append({
            'cen_bf': cen_core.astype(ml_dtypes.bfloat16),
            'mas': mas_core.astype(ml_dtypes.bfloat16),
            'cbf': consts['cbf'],
            'cf32': cf32,
        })
    return in_maps


def run(inputs, trace=False):
    inp = {k: np.asarray(v) for k, v in inputs.items()}
    consts = _prep_consts(inp)
    if 'nc' not in _CACHE:
        _CACHE['nc'] = build_nc()
    nc = _CACHE['nc']
    in_maps = make_in_maps(inp, consts)
    res = run_bass_kernel_spmd(nc, in_maps, list(range(N_CORES)), trace=trace)
    out = np.empty((4, C, H, W), np.float32)
    for core in range(N_CORES):
        b, hf = core // 2, core % 2
        out[b, :, CH * hf:CH * (hf + 1), :] = \
            res.results[core]['out'].astype(np.float32)
    return out, res


def kernel(**inputs):
    return run(inputs)[0]


def bench(inputs, iters=30, reps=0):
    """Time repeated executions with device-resident inputs (no donation)."""
    import time
    import jax
    from jax.sharding import Mesh, PartitionSpec
    from jax.experimental.shard_map import shard_map
    from concourse import bass2jax

    inp = {k: np.asarray(v) for k, v in inputs.items()}
    consts = _prep_consts(inp)
    key = ('nc', reps)
    if key not in _CACHE:
        _CACHE[key] = build_nc(loop_reps=reps)
    nc = _CACHE[key]
    in_maps = make_in_maps(inp, consts)

    bass2jax.install_neuronx_cc_hook()
    in_names, out_names, out_avals, zero_outs = [], [], [], []
    pname = (nc.partition_id_tensor.name if nc.partition_id_tensor else None)
    for alloc in nc.m.functions[0].allocations:
        if not isinstance(alloc, mybir.MemoryLocationSet):
            continue
        name = alloc.memorylocations[0].name
        if alloc.kind == "ExternalInput":
            if name != pname:
                in_names.append(name)
        elif alloc.kind == "ExternalOutput":
            out_names.append(name)
            out_avals.append(jax.core.ShapedArray(
                tuple(alloc.tensor_shape), mybir.dt.np(alloc.dtype)))
            zero_outs.append(np.zeros(tuple(alloc.tensor_shape),
                                      mybir.dt.np(alloc.dtype)))
    n_params = len(in_names)

    def _body(*args):
        operands = list(args)
        all_names = in_names + out_names
        if nc.partition_id_tensor:
            operands.append(bass2jax.partition_id_tensor())
            all_names = all_names + [nc.partition_id_tensor.name]
        outs = bass2jax._bass_exec_p.bind(
            *operands, out_avals=tuple(out_avals), in_names=tuple(all_names),
            out_names=tuple(out_names), lowering_input_output_aliases=(),
            sim_require_finite=True, sim_require_nnan=True, nc=nc)
        return tuple(outs)

    devices = jax.devices()[:N_CORES]
    mesh = Mesh(np.asarray(devices), ("core",))
    nin = n_params + len(out_names)
    sharded = jax.jit(
        shard_map(_body, mesh=mesh,
                  in_specs=(PartitionSpec("core"),) * nin,
                  out_specs=(PartitionSpec("core"),) * len(out_names),
                  check_rep=False),
        donate_argnums=tuple(range(n_params, nin)),
        keep_unused=True,
    )
    concat_in = [np.concatenate([in_maps[c][nm] for c in range(N_CORES)], 0)
                 for nm in in_names]
    concat_zero = [np.zeros((N_CORES * z.shape[0], *z.shape[1:]), z.dtype)
                   for z in zero_outs]
    sh = jax.sharding.NamedSharding(mesh, PartitionSpec("core"))
    dev_in = [jax.device_put(a, sh) for a in concat_in]
    prev = jax.device_put(concat_zero[0], sh)

    outs = sharded(*dev_in, prev)
    jax.block_until_ready(outs)
    result = np.asarray(outs[0]).copy()
    prev = outs[0]
    times = []
    for _ in range(iters):
        t0 = time.perf_counter()
        outs = sharded(*dev_in, prev)
        jax.block_until_ready(outs)
        times.append(time.perf_counter() - t0)
        prev = outs[0]

    full = np.empty((4, C, H, W), np.float32)
    arr = result.reshape(N_CORES, C, CH, W)
    for core in range(N_CORES):
        b, hf = core // 2, core % 2
        full[b, :, CH * hf:CH * (hf + 1), :] = arr[core].astype(np.float32)
    return full, times


# revision 27
# speedup vs baseline: 1.2236x; 1.2236x over previous
"""Trainium2 Bass kernel for nn_ExpansionContrastModule (v2).

Strategy: pure data parallel over 8 cores; each core processes half of one
batch image (128 of 256 rows) with a 3-row halo.

v2 redesign vs v1 baseline:
- cen is loaded ONCE as bf16 and stays resident in SBUF; the final gating
  multiply reads it from SBUF (v1 re-read cen as f32 from HBM: -17MB/core).
- output is written bf16 and converted to f32 on the host (-8MB/core).
- contrast stage uses the difference-product identity
  (x - x_d)(x - x_{-d}) = -D_d(p) * D_d(p - d)  with D_d = x - shift_d(x),
  computed once per direction on an extended domain: 2 DVE ops per
  direction instead of 3 (min/max/sum trees run on negated products with
  swapped/negated weights).
- max(t1,t3) and (t1+t3) are never materialized:
  y = v0*mn + v1m*(t1+t3) + v2*(t1+t3-mn) folds into matmul weights
  (v0-v2)*W for mn and (v1m+v2)*W for t1,t3.
- ~40% of elementwise work + the final multiply run on the Pool engine
  (gpsimd); the final multiply reads the gate straight from PSUM.
- mas 3x3 conv input built with 3 gather-DMAs (overlapping APs) from a
  host-padded [130,258] tensor instead of 18 strided DMAs.

Per-core layout: channels of the reduced tensor x (16) are packed 8 groups
to the 128 SBUF partitions (partition 16g+c = row-group g, channel c).
Two sequential 64-row macro-halves per core.
"""
import dataclasses
import sys

import ml_dtypes
import numpy as np

sys.path.insert(0, "/opt/trn_rl_repo")

import concourse.bass as bass
import concourse.bacc as bacc
import concourse.mybir as mybir
from concourse.tile import TileContext
from concourse.bass_utils import run_bass_kernel_spmd

F32 = mybir.dt.float32
BF = mybir.dt.bfloat16
AF = mybir.ActivationFunctionType
ALU = mybir.AluOpType

N_CORES = 8
C = 128        # input channels
CR = 16        # reduced channels
H = W = 256
CH = 128       # rows per core (half an image)
MH = 2         # macro-halves per core
HB = 64        # rows per macro-half
G = 8          # row-groups per macro-half
GR = 8         # rows per group
XR = GR + 6    # x tile rows (3-row halo each side)
XP = 4         # x tile left/right col pad
XW = W + 2 * XP  # 264
QF = GR * W    # 2048 free elems per macro-half slab
J0 = 3         # x tile row index of the first slab row

BN_EPS = 1e-5

# cbf column blocks
CB_WIN = 0          # [C, 8*C]      w_in block-diag
CB_BC = 1024        # [C, 3*C]      bc conv weights for (mn, t1, t3)
CB_WOUT = 1408      # [C, 8]        w_out block
CB_MAS = 1416       # [72, 8]       mas 3x3 kernel
CB_BCAST = 1424     # [40, 8*C]     gate broadcast (ct_g + ge2_{32+g} per group)
CB_EYE = 2448       # [C, 3*C]      -w1m*I, -w2*I, -w0*I (t_s via PE)
NCB = 2832

# cf32 scalar columns
S_BIN, S_BTOP, S_BBOT, S_BNS, S_BNB = 0, 1, 2, 3, 4
S_NW0, S_NW1_4, S_NW2 = 5, 6, 7
S_BT0, S_BT1 = 8, 9
S_BOUT, S_MB1, S_MW2, S_MB2 = 12, 13, 14, 15
NCF = 16

_CACHE = {}


def _fl(t):
    """Flatten the two free dims of a [P, a, b] tile AP into [P, a*b]."""
    return t[:, :, :].rearrange("p a b -> p (a b)")


def build_nc(loop_reps=0):
    nc = bacc.Bacc("TRN2", target_bir_lowering=False, debug=False,
                   num_devices=N_CORES)
    cen_d = nc.dram_tensor("cen_bf", [C, CH + 6, W], BF, kind="ExternalInput")
    mas_d = nc.dram_tensor("mas", [CH + 2, W + 2], BF, kind="ExternalInput")
    cbf_d = nc.dram_tensor("cbf", [C, NCB], BF, kind="ExternalInput")
    cf32_d = nc.dram_tensor("cf32", [C, NCF], F32, kind="ExternalInput")
    out_d = nc.dram_tensor("out", [C, CH, W], BF, kind="ExternalOutput")

    with TileContext(nc) as tc:
        import contextlib
        _stk = contextlib.ExitStack()
        with _stk:
            ep = _stk.enter_context
            cpool = ep(tc.tile_pool(name="const", bufs=1))
            cenpool = ep(tc.tile_pool(name="cen", bufs=1))
            xpool = ep(tc.tile_pool(name="x", bufs=2))
            xopool = ep(tc.tile_pool(name="xo", bufs=2))
            dpool = ep(tc.tile_pool(name="d", bufs=3))
            dopool = ep(tc.tile_pool(name="do", bufs=1))
            upool = ep(tc.tile_pool(name="u", bufs=4))
            trpool = ep(tc.tile_pool(name="tr", bufs=3))
            tmppool = ep(tc.tile_pool(name="tm", bufs=2))
            tpool = ep(tc.tile_pool(name="t", bufs=3))
            zpool = ep(tc.tile_pool(name="z", bufs=1))
            gpool = ep(tc.tile_pool(name="g", bufs=1))
            mpool = ep(tc.tile_pool(name="m", bufs=1))
            m9pool = ep(tc.tile_pool(name="m9", bufs=1))
            opool = ep(tc.tile_pool(name="o", bufs=2))
            pxpool = ep(tc.tile_pool(name="px", bufs=2, space="PSUM"))
            smpool = ep(tc.tile_pool(name="sm", bufs=2, space="PSUM"))
            pgpool = ep(tc.tile_pool(name="pg", bufs=2, space="PSUM"))

            # ---- constants ----
            cbf_sb = cpool.tile([C, NCB], BF, tag="c_bf")
            cf32_sb = cpool.tile([C, NCF], F32, tag="c_f32")
            nc.sync.dma_start(out=cbf_sb[:], in_=cbf_d[:])
            nc.sync.dma_start(out=cf32_sb[:], in_=cf32_d[:])

            def sc(col, p=C):
                return cf32_sb[0:p, col:col + 1]

            def stage_A(mh, cen_fl, x, with_mas9=True):
                """mas9 gather (opt), x conv, halos, x_odd. PE/Act/SP work."""
                base = mh * HB
                mas9 = None
                if with_mas9:
                    mas9 = emit_mas9(mh)
                for c2 in range(4):
                    px = pxpool.tile([C, 512], F32, tag="px")
                    for g in range(G):
                        r0 = (base + 8 * g + J0 + 2 * c2) * W
                        nc.tensor.matmul(
                            px[:], cbf_sb[:, CB_WIN + g * C:CB_WIN + (g + 1) * C],
                            cen_fl[:, r0:r0 + 512],
                            start=(g == 0), stop=(g == G - 1))
                    nc.scalar.activation(
                        x[:, J0 + 2 * c2:J0 + 2 * c2 + 2, XP:XP + W],
                        px[:].rearrange("p (a b) -> p a b", a=2),
                        AF.Identity, bias=sc(S_BIN), scale=1.0)
                # edge rows: top (group 0 only), bottom (group 7 only)
                bt_c = S_BTOP if mh == 0 else S_BIN
                bb_c = S_BBOT if mh == MH - 1 else S_BIN
                for j in range(3):
                    pe = pxpool.tile([C, 512], F32, tag="px")
                    nc.tensor.matmul(pe[:, 0:W], cbf_sb[:, CB_WIN:CB_WIN + C],
                                     cen_fl[:, (base + j) * W:(base + j + 1) * W],
                                     start=True, stop=True)
                    nc.scalar.activation(
                        x[0:CR, j, XP:XP + W], pe[0:CR, 0:W], AF.Identity,
                        bias=sc(bt_c, CR), scale=1.0)
                    pe2 = pxpool.tile([C, 512], F32, tag="px")
                    nc.tensor.matmul(
                        pe2[:, 0:W], cbf_sb[:, CB_WIN + 7 * C:CB_WIN + 8 * C],
                        cen_fl[:, (base + 56 + 11 + j) * W:(base + 56 + 12 + j) * W],
                        start=True, stop=True)
                    # start partition must be a multiple of 32; rows 96:112 get
                    # junk and are re-written by the halo DMA below.
                    nc.scalar.activation(
                        x[96:C, 11 + j, XP:XP + W], pe2[96:C, 0:W],
                        AF.Identity, bias=cf32_sb[96:C, bb_c:bb_c + 1], scale=1.0)
                # interior halos between groups via partition-shifted SBUF DMA
                nc.sync.dma_start(out=x[CR:C, 0:3, XP:XP + W],
                                  in_=x[0:C - CR, GR:GR + 3, XP:XP + W])
                nc.sync.dma_start(out=x[0:C - CR, GR + 3:GR + 6, XP:XP + W],
                                  in_=x[CR:C, 3:6, XP:XP + W])
                # x_odd[c] = x[c+1]: one flat shifted copy (Act)
                x_odd = xopool.tile([C, XR, XW], BF, tag="xo")
                nc.scalar.copy(_fl(x_odd)[:, 0:XR * XW - 1], _fl(x)[:, 1:XR * XW])
                return {'x': x, 'xo': x_odd, 'mas9': mas9}

            def emit_mas9(mh):
                """3 overlapping-AP gather DMAs building the 9-shift layout."""
                base = mh * HB
                mas9 = m9pool.tile([72, GR, W], BF, tag="m9")
                msrc = mas_d[:]
                for dy in range(3):
                    apd = dataclasses.replace(
                        msrc, offset=(base + dy) * (W + 2),
                        ap=[[1, 3], [GR * (W + 2), G], [W + 2, GR], [1, W]])
                    nc.sync.dma_start(out=mas9[24 * dy:24 * (dy + 1)], in_=apd)
                return mas9

            def stage_B(st):
                """mas conv part 1: PE matmuls + Act silu (exact, via LUT)."""
                m9f = _fl(st['mas9'])
                msl = mpool.tile([G, QF], BF, tag="msl")
                for c2 in range(4):
                    pm = smpool.tile([C, 512], F32, tag="sm")
                    nc.tensor.matmul(pm[0:G, :], cbf_sb[0:72, CB_MAS:CB_MAS + 8],
                                     m9f[:, 512 * c2:512 * (c2 + 1)],
                                     start=True, stop=True)
                    nc.scalar.activation(msl[:, 512 * c2:512 * (c2 + 1)],
                                         pm[0:G, :], AF.Silu,
                                         bias=sc(S_MB1, G), scale=1.0)
                st['msl'] = msl

            def stage_C(st):
                """Contrast stage: DVE-heavy with Pool offload."""
                x, x_odd = st['x'], st['xo']
                t_tiles = []
                for s in (1, 3):
                    nr = GR + s
                    wd = W + s + 1  # even extended width for diag/col dirs
                    Db = dpool.tile([C, 11, XW], BF, tag="d")
                    nc.vector.tensor_tensor(
                        Db[:, 0:nr, 4:4 + W], x[:, J0:J0 + nr, XP:XP + W],
                        x[:, J0 - s:J0 - s + nr, XP:XP + W], ALU.subtract)
                    Da = dpool.tile([C, 11, XW], BF, tag="d")
                    nc.vector.tensor_tensor(
                        Da[:, 0:nr, 4:4 + wd], x[:, J0:J0 + nr, XP:XP + wd],
                        x_odd[:, J0 - s:J0 - s + nr, XP - s - 1:XP - s - 1 + wd],
                        ALU.subtract)
                    Doa = dopool.tile([C, GR, W], BF, tag="do")
                    nc.scalar.copy(
                        Doa[:], Da[:, s:s + GR, s + 4:s + 4 + W])
                    ub = upool.tile([C, GR, W], BF, tag="u")
                    nc.vector.tensor_tensor(
                        ub[:], Db[:, 0:GR, 4:4 + W], Db[:, s:s + GR, 4:4 + W],
                        ALU.mult)
                    ua = upool.tile([C, GR, W], BF, tag="u")
                    nc.vector.tensor_tensor(
                        ua[:], Da[:, 0:GR, 4:4 + W], Doa[:], ALU.mult)
                    m01 = trpool.tile([C, QF], BF, tag="tr")
                    nc.vector.tensor_tensor(m01[:], _fl(ua), _fl(ub), ALU.min)
                    M01 = trpool.tile([C, QF], BF, tag="tr")
                    nc.vector.tensor_tensor(M01[:], _fl(ua), _fl(ub), ALU.max)

                    Dc = dpool.tile([C, 11, XW], BF, tag="d")
                    nc.vector.tensor_tensor(
                        Dc[:, 0:nr, 4:4 + wd],
                        x_odd[:, J0:J0 + nr, XP - s - 1:XP - s - 1 + wd],
                        x[:, J0 - s:J0 - s + nr, XP:XP + wd], ALU.subtract)
                    Doc = dopool.tile([C, GR, W], BF, tag="do")
                    nc.scalar.copy(
                        Doc[:], Dc[:, 0:GR, s + 4:s + 4 + W])
                    uc = upool.tile([C, GR, W], BF, tag="u")
                    nc.vector.tensor_tensor(
                        uc[:], Doc[:], Dc[:, s:s + GR, 4:4 + W], ALU.mult)
                    Dd = dpool.tile([C, 11, XW], BF, tag="d")
                    nc.vector.tensor_tensor(
                        Dd[:, 0:GR, 4:4 + wd], x[:, J0:J0 + GR, XP:XP + wd],
                        x_odd[:, J0:J0 + GR, XP - s - 1:XP - s - 1 + wd],
                        ALU.subtract)
                    Dod = dopool.tile([C, GR, W], BF, tag="do")
                    nc.scalar.copy(
                        Dod[:], Dd[:, 0:GR, s + 4:s + 4 + W])
                    ud = upool.tile([C, GR, W], BF, tag="u")
                    nc.vector.tensor_tensor(
                        ud[:], Dd[:, 0:GR, 4:4 + W], Dod[:], ALU.mult)

                    m23 = trpool.tile([C, QF], BF, tag="tr")
                    nc.vector.tensor_tensor(m23[:], _fl(uc), _fl(ud), ALU.min)
                    min4 = tmppool.tile([C, QF], BF, tag="tmp")
                    nc.vector.tensor_tensor(min4[:], m01[:], m23[:], ALU.min)
                    M23 = trpool.tile([C, QF], BF, tag="tr")
                    nc.vector.tensor_tensor(M23[:], _fl(uc), _fl(ud), ALU.max)
                    max4 = tmppool.tile([C, QF], BF, tag="tmp")
                    nc.vector.tensor_tensor(max4[:], M01[:], M23[:], ALU.max)

                    # t_s = -(w0*max4 + w1m*(ua+ub+uc+ud) + w2*min4): pure
                    # linear combine -> 6 accumulating PE matmuls per chunk
                    # with pre-scaled identity weights, evacuated by Act.
                    t_s = tpool.tile([C, QF], BF, tag="ts")
                    ufl = [_fl(ua), _fl(ub), _fl(uc), _fl(ud)]
                    for c2 in range(4):
                        cc = slice(512 * c2, 512 * (c2 + 1))
                        pt = pxpool.tile([C, 512], F32, tag="px")
                        for i, uf in enumerate(ufl):
                            nc.tensor.matmul(
                                pt[:], cbf_sb[:, CB_EYE:CB_EYE + C],
                                uf[:, cc], start=(i == 0), stop=False)
                        nc.tensor.matmul(pt[:],
                                         cbf_sb[:, CB_EYE + C:CB_EYE + 2 * C],
                                         min4[:, cc], start=False, stop=False)
                        nc.tensor.matmul(pt[:],
                                         cbf_sb[:, CB_EYE + 2 * C:CB_EYE + 3 * C],
                                         max4[:, cc], start=False, stop=True)
                        nc.scalar.activation(t_s[:, cc], pt[:], AF.Identity,
                                             bias=0.0, scale=1.0)
                    t_tiles.append(t_s)

                t1, t3 = t_tiles
                mn = tpool.tile([C, QF], BF, tag="ts")
                nc.vector.tensor_tensor(mn[:], t1[:], t3[:], ALU.min)
                st['t1'], st['t3'], st['mn'] = t1, t3, mn

            def stage_D(mh, st, cen_sb, gate16):
                """bc+bn+silu, w_out, gate combine (Pool), bcast+final, out."""
                base = mh * HB
                t1, t3, mn = st['t1'], st['t3'], st['mn']
                zq = zpool.tile([C, QF], BF, tag="zq")
                for c2 in range(4):
                    cc = slice(512 * c2, 512 * (c2 + 1))
                    pb = pxpool.tile([C, 512], F32, tag="px")
                    nc.tensor.matmul(pb[:], cbf_sb[:, CB_BC:CB_BC + C],
                                     mn[:, cc], start=True, stop=False)
                    nc.tensor.matmul(pb[:], cbf_sb[:, CB_BC + C:CB_BC + 2 * C],
                                     t1[:, cc], start=False, stop=False)
                    nc.tensor.matmul(pb[:], cbf_sb[:, CB_BC + 2 * C:CB_BC + 3 * C],
                                     t3[:, cc], start=False, stop=True)
                    nc.scalar.activation(zq[:, cc], pb[:], AF.Silu,
                                         bias=sc(S_BNB), scale=sc(S_BNS))

                om_q = gpool.tile([G, QF], BF, tag="om")
                for c2 in range(4):
                    pu = smpool.tile([C, 512], F32, tag="sm")
                    nc.tensor.matmul(pu[0:G, :], cbf_sb[:, CB_WOUT:CB_WOUT + 8],
                                     zq[:, 512 * c2:512 * (c2 + 1)],
                                     start=True, stop=True)
                    nc.scalar.activation(om_q[:, 512 * c2:512 * (c2 + 1)],
                                         pu[0:G, :], AF.Tanh,
                                         bias=sc(S_BOUT, G), scale=0.5)

                # h_mm at base partition 0 (HW: SBUF STT operands must share
                # base partition), then copied to gate40 rows 32:40 (weight
                # s1p); row 8 is constant 1.0 (weight s3p); rows 0:8 =
                # (h_mm+s0p/s2p)*h_om (weight s2p).
                mm_q = mpool.tile([G, QF], BF, tag="mm")
                nc.scalar.activation(mm_q[:], st['msl'][:], AF.Tanh,
                                     bias=sc(S_MB2, G), scale=sc(S_MW2, G))
                nc.vector.tensor_copy(gate16[32:40, :], mm_q[:])
                bt = mpool.tile([G, QF], BF, tag="bt")
                nc.scalar.activation(bt[:], mm_q[:], AF.Identity,
                                     bias=sc(S_BT0, G), scale=sc(S_BT1, G))
                nc.vector.tensor_tensor(gate16[0:G, :], om_q[:], bt[:], ALU.mult)

                # final multiply: GPSIMD cannot read PSUM on HW, so half the
                # groups multiply on DVE straight from PSUM (fp32, 1x) and
                # half go Act-evac (bf16) -> Pool multiply.
                for g in range(G):
                    out_t = opool.tile([C, GR, W], BF, tag="o")
                    lh = cbf_sb[0:40, CB_BCAST + g * C:CB_BCAST + (g + 1) * C]
                    for h in range(2):
                        pg = pgpool.tile([C, 1024], F32, tag="pg")
                        for c4 in range(2):
                            c2 = 2 * h + c4
                            nc.tensor.matmul(
                                pg[:, 512 * c4:512 * (c4 + 1)], lh,
                                gate16[:, 512 * c2:512 * (c2 + 1)],
                                start=True, stop=True)
                        r0 = base + 8 * g + 4 * h
                        cs = cen_sb[:, 3 + r0:3 + r0 + 4, :]
                        ot = out_t[:, 4 * h:4 * h + 4, :]
                        pgv = pg[:].rearrange("p (a b) -> p a b", a=4)
                        if g % 2 == 0:
                            nc.vector.tensor_tensor(ot, cs, pgv, ALU.mult)
                        else:
                            gs = trpool.tile([C, QF], BF, tag="tr")
                            gsv = gs[:, 0:1024].rearrange(
                                "p (a b) -> p a b", a=4)
                            nc.scalar.activation(gsv, pgv, AF.Identity,
                                                 bias=0.0, scale=1.0)
                            nc.vector.tensor_tensor(ot, cs, gsv, ALU.mult)
                    eng = nc.scalar if g % 4 == 3 else nc.sync
                    eng.dma_start(
                        out=out_d[:, base + 8 * g:base + 8 * g + GR, :],
                        in_=out_t[:])

            def emit_all():
                # PE p-state warmup: dummy matmuls so the clock ramps while
                # cen streams in (results never read)
                pw = pxpool.tile([C, 512], F32, tag="px")
                for i in range(10):
                    nc.tensor.matmul(pw[:], cbf_sb[:, 0:C],
                                     cbf_sb[:, 0:512], start=(i == 0),
                                     stop=(i == 9))
                cen_sb = cenpool.tile([C, CH + 6, W], BF, tag="cen")
                nrows = [18, 18, 17, 17, 32, 32]
                r = 0
                for i, n in enumerate(nrows):
                    nc.sync.dma_start(out=cen_sb[:, r:r + n, :],
                                      in_=cen_d[:, r:r + n, :])
                    r += n
                cen_fl = _fl(cen_sb)
                gate16 = gpool.tile([40, QF], BF, tag="g16")
                nc.gpsimd.memset(gate16[0:32, :], 1.0)
                # x ring buffers allocated up front so their constant zero
                # col-pads are set once, off the critical path
                xs = []
                for _ in range(MH):
                    xt = xpool.tile([C, XR, XW], BF, tag="x")
                    nc.gpsimd.memset(xt[:, :, 0:XP], 0.0)
                    nc.gpsimd.memset(xt[:, :, W + XP:XW], 0.0)
                    xs.append(xt)
                # software-pipelined over the two macro-halves
                st0 = stage_A(0, cen_fl, xs[0])
                stage_B(st0)
                mas9_1 = emit_mas9(1)
                stage_C(st0)
                st1 = stage_A(1, cen_fl, xs[1], with_mas9=False)
                st1['mas9'] = mas9_1
                stage_D(0, st0, cen_sb, gate16)
                stage_B(st1)
                stage_C(st1)
                stage_D(1, st1, cen_sb, gate16)

            rep_ctx = (tc.For_i(0, loop_reps, 1) if loop_reps
                       else contextlib.nullcontext())
            with rep_ctx:
                emit_all()
    nc.compile()
    return nc


def _softmax(v):
    e = np.exp(v - v.max())
    return e / e.sum()


def _prep_consts(inp):
    w = _softmax(inp['scale1'])
    v = _softmax(inp['scale2'])
    s3 = _softmax(inp['scale3'])
    inv = inp['bn_gamma'] / np.sqrt(inp['bn_var'] + BN_EPS)
    bnb = inp['bn_beta'] - inp['bn_mean'] * inv

    w_in_blk = np.zeros((C, G, C), np.float32)
    for g in range(G):
        w_in_blk[:, g, CR * g:CR * (g + 1)] = inp['w_in'].T
    bc_blk = np.kron(np.eye(G), inp['bc_w'].T)
    A, B = v[0] - v[2], v[1] / 2.0 + v[2]
    wout_lhsT = np.kron(np.eye(G), inp['w_out'][0][:, None]).astype(np.float32)
    k_flat = inp['mas_w1'][0, 0].reshape(9)
    mas_lhsT = np.kron(k_flat[:, None], np.eye(G)).astype(np.float32)
    # gate in tanh basis: om = (1+h_om)/2, mm = (1+h_mm)/2 with
    # h = tanh(pre/2); gate = s3p + s0p*h_om + s1p*h_mm + s2p*h_om*h_mm
    s0p = s3[0] / 2 + s3[2] / 4
    s1p = s3[1] / 2 + s3[2] / 4
    s2p = s3[2] / 4
    s3p = s3[3] + s3[0] / 2 + s3[1] / 2 + s3[2] / 4
    bcast = np.zeros((40, G, C), np.float32)
    for g in range(G):
        bcast[g, g, :] = 1.0
        bcast[8, g, :] = s3p
        bcast[32 + g, g, :] = s1p

    cbf = np.zeros((C, NCB), np.float32)
    cbf[:, CB_WIN:CB_WIN + G * C] = w_in_blk.reshape(C, G * C)
    cbf[:, CB_BC:CB_BC + C] = A * bc_blk
    cbf[:, CB_BC + C:CB_BC + 2 * C] = B * bc_blk
    cbf[:, CB_BC + 2 * C:CB_BC + 3 * C] = B * bc_blk
    cbf[:, CB_WOUT:CB_WOUT + 8] = wout_lhsT
    cbf[0:72, CB_MAS:CB_MAS + 8] = mas_lhsT
    cbf[0:40, CB_BCAST:CB_BCAST + G * C] = bcast.reshape(40, G * C)
    eye = np.eye(C, dtype=np.float32)
    cbf[:, CB_EYE:CB_EYE + C] = (-w[1] / 4.0) * eye
    cbf[:, CB_EYE + C:CB_EYE + 2 * C] = -w[2] * eye
    cbf[:, CB_EYE + 2 * C:CB_EYE + 3 * C] = -w[0] * eye

    b_in_t = np.tile(inp['b_in'], G)[:, None].astype(np.float32)
    cf32 = np.zeros((C, NCF), np.float32)
    cf32[:, S_BIN:S_BIN + 1] = b_in_t
    cf32[:, S_BNS:S_BNS + 1] = np.tile(inv, G)[:, None]
    cf32[:, S_BNB:S_BNB + 1] = np.tile(bnb, G)[:, None]
    scal = [(S_NW0, -w[0]), (S_NW1_4, -w[1] / 4.0), (S_NW2, -w[2]),
            (S_BT0, s0p), (S_BT1, s2p),
            (S_BOUT, inp['b_out'][0] / 2.0), (S_MB1, inp['mas_b1'][0]),
            (S_MW2, inp['mas_w2'][0, 0] / 2.0),
            (S_MB2, inp['mas_b2'][0] / 2.0)]
    for col, val in scal:
        cf32[:, col] = val
    return {'cbf': cbf.astype(ml_dtypes.bfloat16), 'cf32': cf32,
            'b_in_t': b_in_t}


def make_in_maps(inp, consts):
    in_maps = []
    for core in range(N_CORES):
        b, hf = core // 2, core % 2
        r0 = CH * hf
        cen_pad = np.pad(inp['cen'][b], ((0, 0), (3, 3), (0, 0)))
        cen_core = np.ascontiguousarray(cen_pad[:, r0:r0 + CH + 6, :])
        mas_pad = np.pad(inp['mas'][b, 0], ((1, 1), (1, 1)))
        mas_core = np.ascontiguousarray(mas_pad[r0:r0 + CH + 2, :])
        cf32 = consts['cf32'].copy()
        if hf != 0:
            cf32[:, S_BTOP:S_BTOP + 1] = consts['b_in_t']
        if hf != 1:
            cf32[:, S_BBOT:S_BBOT + 1] = consts['b_in_t']
        in_maps.append({
            'cen_bf': cen_core.astype(ml_dtypes.bfloat16),
            'mas': mas_core.astype(ml_dtypes.bfloat16),
            'cbf': consts['cbf'],
            'cf32': cf32,
        })
    return in_maps


def run(inputs, trace=False):
    inp = {k: np.asarray(v) for k, v in inputs.items()}
    consts = _prep_consts(inp)
    if 'nc' not in _CACHE:
        _CACHE['nc'] = build_nc()
    nc = _CACHE['nc']
    in_maps = make_in_maps(inp, consts)
    res = run_bass_kernel_spmd(nc, in_maps, list(range(N_CORES)), trace=trace)
    out = np.empty((4, C, H, W), np.float32)
    for core in range(N_CORES):
        b, hf = core // 2, core % 2
        out[b, :, CH * hf:CH * (hf + 1), :] = \
            res.results[core]['out'].astype(np.float32)
    return out, res


def kernel(**inputs):
    return run(inputs)[0]


def bench(inputs, iters=30, reps=0):
    """Time repeated executions with device-resident inputs (no donation)."""
    import time
    import jax
    from jax.sharding import Mesh, PartitionSpec
    from jax.experimental.shard_map import shard_map
    from concourse import bass2jax

    inp = {k: np.asarray(v) for k, v in inputs.items()}
    consts = _prep_consts(inp)
    key = ('nc', reps)
    if key not in _CACHE:
        _CACHE[key] = build_nc(loop_reps=reps)
    nc = _CACHE[key]
    in_maps = make_in_maps(inp, consts)

    bass2jax.install_neuronx_cc_hook()
    in_names, out_names, out_avals, zero_outs = [], [], [], []
    pname = (nc.partition_id_tensor.name if nc.partition_id_tensor else None)
    for alloc in nc.m.functions[0].allocations:
        if not isinstance(alloc, mybir.MemoryLocationSet):
            continue
        name = alloc.memorylocations[0].name
        if alloc.kind == "ExternalInput":
            if name != pname:
                in_names.append(name)
        elif alloc.kind == "ExternalOutput":
            out_names.append(name)
            out_avals.append(jax.core.ShapedArray(
                tuple(alloc.tensor_shape), mybir.dt.np(alloc.dtype)))
            zero_outs.append(np.zeros(tuple(alloc.tensor_shape),
                                      mybir.dt.np(alloc.dtype)))
    n_params = len(in_names)

    def _body(*args):
        operands = list(args)
        all_names = in_names + out_names
        if nc.partition_id_tensor:
            operands.append(bass2jax.partition_id_tensor())
            all_names = all_names + [nc.partition_id_tensor.name]
        outs = bass2jax._bass_exec_p.bind(
            *operands, out_avals=tuple(out_avals), in_names=tuple(all_names),
            out_names=tuple(out_names), lowering_input_output_aliases=(),
            sim_require_finite=True, sim_require_nnan=True, nc=nc)
        return tuple(outs)

    devices = jax.devices()[:N_CORES]
    mesh = Mesh(np.asarray(devices), ("core",))
    nin = n_params + len(out_names)
    sharded = jax.jit(
        shard_map(_body, mesh=mesh,
                  in_specs=(PartitionSpec("core"),) * nin,
                  out_specs=(PartitionSpec("core"),) * len(out_names),
                  check_rep=False),
        donate_argnums=tuple(range(n_params, nin)),
        keep_unused=True,
    )
    concat_in = [np.concatenate([in_maps[c][nm] for c in range(N_CORES)], 0)
                 for nm in in_names]
    concat_zero = [np.zeros((N_CORES * z.shape[0], *z.shape[1:]), z.dtype)
                   for z in zero_outs]
    sh = jax.sharding.NamedSharding(mesh, PartitionSpec("core"))
    dev_in = [jax.device_put(a, sh) for a in concat_in]
    prev = jax.device_put(concat_zero[0], sh)

    outs = sharded(*dev_in, prev)
    jax.block_until_ready(outs)
    result = np.asarray(outs[0]).copy()
    prev = outs[0]
    times = []
    for _ in range(iters):
        t0 = time.perf_counter()
        outs = sharded(*dev_in, prev)
        jax.block_until_ready(outs)
        times.append(time.perf_counter() - t0)
        prev = outs[0]

    full = np.empty((4, C, H, W), np.float32)
    arr = result.reshape(N_CORES, C, CH, W)
    for core in range(N_CORES):
        b, hf = core // 2, core % 2
        full[b, :, CH * hf:CH * (hf + 1), :] = arr[core].astype(np.float32)
    return full, times


# revision 28
# speedup vs baseline: 1.2633x; 1.0325x over previous
"""Trainium2 Bass kernel for nn_ExpansionContrastModule (v2).

Strategy: pure data parallel over 8 cores; each core processes half of one
batch image (128 of 256 rows) with a 3-row halo.

v2 redesign vs v1 baseline:
- cen is loaded ONCE as bf16 and stays resident in SBUF; the final gating
  multiply reads it from SBUF (v1 re-read cen as f32 from HBM: -17MB/core).
- output is written bf16 and converted to f32 on the host (-8MB/core).
- contrast stage uses the difference-product identity
  (x - x_d)(x - x_{-d}) = -D_d(p) * D_d(p - d)  with D_d = x - shift_d(x),
  computed once per direction on an extended domain: 2 DVE ops per
  direction instead of 3 (min/max/sum trees run on negated products with
  swapped/negated weights).
- max(t1,t3) and (t1+t3) are never materialized:
  y = v0*mn + v1m*(t1+t3) + v2*(t1+t3-mn) folds into matmul weights
  (v0-v2)*W for mn and (v1m+v2)*W for t1,t3.
- ~40% of elementwise work + the final multiply run on the Pool engine
  (gpsimd); the final multiply reads the gate straight from PSUM.
- mas 3x3 conv input built with 3 gather-DMAs (overlapping APs) from a
  host-padded [130,258] tensor instead of 18 strided DMAs.

Per-core layout: channels of the reduced tensor x (16) are packed 8 groups
to the 128 SBUF partitions (partition 16g+c = row-group g, channel c).
Two sequential 64-row macro-halves per core.
"""
import dataclasses
import sys

import ml_dtypes
import numpy as np

sys.path.insert(0, "/opt/trn_rl_repo")

import concourse.bass as bass
import concourse.bacc as bacc
import concourse.mybir as mybir
from concourse.tile import TileContext
from concourse.bass_utils import run_bass_kernel_spmd

F32 = mybir.dt.float32
BF = mybir.dt.bfloat16
AF = mybir.ActivationFunctionType
ALU = mybir.AluOpType

N_CORES = 8
C = 128        # input channels
CR = 16        # reduced channels
H = W = 256
CH = 128       # rows per core (half an image)
MH = 2         # macro-halves per core
HB = 64        # rows per macro-half
G = 8          # row-groups per macro-half
GR = 8         # rows per group
XR = GR + 6    # x tile rows (3-row halo each side)
XP = 4         # x tile left/right col pad
XW = W + 2 * XP  # 264
QF = GR * W    # 2048 free elems per macro-half slab
J0 = 3         # x tile row index of the first slab row

BN_EPS = 1e-5

# cbf column blocks
CB_WIN = 0          # [C, 8*C]      w_in block-diag
CB_BC = 1024        # [C, 3*C]      bc conv weights for (mn, t1, t3)
CB_WOUT = 1408      # [C, 8]        w_out block
CB_MAS = 1416       # [72, 8]       mas 3x3 kernel
CB_BCAST = 1424     # [40, 8*C]     gate broadcast (ct_g + ge2_{32+g} per group)
CB_EYE = 2448       # [C, 3*C]      -w1m*I, -w2*I, -w0*I (t_s via PE)
NCB = 2832

# cf32 scalar columns
S_BIN, S_BTOP, S_BBOT, S_BNS, S_BNB = 0, 1, 2, 3, 4
S_NW0, S_NW1_4, S_NW2 = 5, 6, 7
S_BT0, S_BT1 = 8, 9
S_BOUT, S_MB1, S_MW2, S_MB2 = 12, 13, 14, 15
NCF = 16

_CACHE = {}


def _fl(t):
    """Flatten the two free dims of a [P, a, b] tile AP into [P, a*b]."""
    return t[:, :, :].rearrange("p a b -> p (a b)")


def build_nc(loop_reps=0):
    nc = bacc.Bacc("TRN2", target_bir_lowering=False, debug=False,
                   num_devices=N_CORES)
    cen_d = nc.dram_tensor("cen_bf", [C, CH + 6, W], BF, kind="ExternalInput")
    mas_d = nc.dram_tensor("mas", [CH + 2, W + 2], BF, kind="ExternalInput")
    cbf_d = nc.dram_tensor("cbf", [C, NCB], BF, kind="ExternalInput")
    cf32_d = nc.dram_tensor("cf32", [C, NCF], F32, kind="ExternalInput")
    out_d = nc.dram_tensor("out", [C, CH, W], BF, kind="ExternalOutput")

    with TileContext(nc) as tc:
        import contextlib
        _stk = contextlib.ExitStack()
        with _stk:
            ep = _stk.enter_context
            cpool = ep(tc.tile_pool(name="const", bufs=1))
            cenpool = ep(tc.tile_pool(name="cen", bufs=1))
            xpool = ep(tc.tile_pool(name="x", bufs=2))
            xopool = ep(tc.tile_pool(name="xo", bufs=2))
            dpool = ep(tc.tile_pool(name="d", bufs=3))
            dopool = ep(tc.tile_pool(name="do", bufs=1))
            upool = ep(tc.tile_pool(name="u", bufs=4))
            trpool = ep(tc.tile_pool(name="tr", bufs=3))
            tmppool = ep(tc.tile_pool(name="tm", bufs=2))
            tpool = ep(tc.tile_pool(name="t", bufs=3))
            zpool = ep(tc.tile_pool(name="z", bufs=1))
            gpool = ep(tc.tile_pool(name="g", bufs=1))
            mpool = ep(tc.tile_pool(name="m", bufs=1))
            m9pool = ep(tc.tile_pool(name="m9", bufs=1))
            opool = ep(tc.tile_pool(name="o", bufs=2))
            pxpool = ep(tc.tile_pool(name="px", bufs=2, space="PSUM"))
            smpool = ep(tc.tile_pool(name="sm", bufs=2, space="PSUM"))
            pgpool = ep(tc.tile_pool(name="pg", bufs=2, space="PSUM"))

            # ---- constants ----
            cbf_sb = cpool.tile([C, NCB], BF, tag="c_bf")
            cf32_sb = cpool.tile([C, NCF], F32, tag="c_f32")
            nc.sync.dma_start(out=cbf_sb[:], in_=cbf_d[:])
            nc.sync.dma_start(out=cf32_sb[:], in_=cf32_d[:])

            def sc(col, p=C):
                return cf32_sb[0:p, col:col + 1]

            def stage_A(mh, cen_fl, x, with_mas9=True):
                """mas9 gather (opt), x conv, halos, x_odd. PE/Act/SP work."""
                base = mh * HB
                mas9 = None
                if with_mas9:
                    mas9 = emit_mas9(mh)
                for c2 in range(4):
                    px = pxpool.tile([C, 512], F32, tag="px")
                    for g in range(G):
                        r0 = (base + 8 * g + J0 + 2 * c2) * W
                        nc.tensor.matmul(
                            px[:], cbf_sb[:, CB_WIN + g * C:CB_WIN + (g + 1) * C],
                            cen_fl[:, r0:r0 + 512],
                            start=(g == 0), stop=(g == G - 1))
                    nc.scalar.activation(
                        x[:, J0 + 2 * c2:J0 + 2 * c2 + 2, XP:XP + W],
                        px[:].rearrange("p (a b) -> p a b", a=2),
                        AF.Identity, bias=sc(S_BIN), scale=1.0)
                # edge rows: top (group 0 only), bottom (group 7 only)
                bt_c = S_BTOP if mh == 0 else S_BIN
                bb_c = S_BBOT if mh == MH - 1 else S_BIN
                for j in range(3):
                    pe = pxpool.tile([C, 512], F32, tag="px")
                    nc.tensor.matmul(pe[:, 0:W], cbf_sb[:, CB_WIN:CB_WIN + C],
                                     cen_fl[:, (base + j) * W:(base + j + 1) * W],
                                     start=True, stop=True)
                    nc.scalar.activation(
                        x[0:CR, j, XP:XP + W], pe[0:CR, 0:W], AF.Identity,
                        bias=sc(bt_c, CR), scale=1.0)
                    pe2 = pxpool.tile([C, 512], F32, tag="px")
                    nc.tensor.matmul(
                        pe2[:, 0:W], cbf_sb[:, CB_WIN + 7 * C:CB_WIN + 8 * C],
                        cen_fl[:, (base + 56 + 11 + j) * W:(base + 56 + 12 + j) * W],
                        start=True, stop=True)
                    # start partition must be a multiple of 32; rows 96:112 get
                    # junk and are re-written by the halo DMA below.
                    nc.scalar.activation(
                        x[96:C, 11 + j, XP:XP + W], pe2[96:C, 0:W],
                        AF.Identity, bias=cf32_sb[96:C, bb_c:bb_c + 1], scale=1.0)
                # interior halos between groups via partition-shifted SBUF DMA
                nc.sync.dma_start(out=x[CR:C, 0:3, XP:XP + W],
                                  in_=x[0:C - CR, GR:GR + 3, XP:XP + W])
                nc.sync.dma_start(out=x[0:C - CR, GR + 3:GR + 6, XP:XP + W],
                                  in_=x[CR:C, 3:6, XP:XP + W])
                # x_odd[c] = x[c+1]: one flat shifted copy (Act)
                x_odd = xopool.tile([C, XR, XW], BF, tag="xo")
                nc.scalar.copy(_fl(x_odd)[:, 0:XR * XW - 1], _fl(x)[:, 1:XR * XW])
                return {'x': x, 'xo': x_odd, 'mas9': mas9}

            def emit_mas9(mh):
                """3 overlapping-AP gather DMAs building the 9-shift layout."""
                base = mh * HB
                mas9 = m9pool.tile([72, GR, W], BF, tag="m9")
                msrc = mas_d[:]
                for dy in range(3):
                    apd = dataclasses.replace(
                        msrc, offset=(base + dy) * (W + 2),
                        ap=[[1, 3], [GR * (W + 2), G], [W + 2, GR], [1, W]])
                    nc.sync.dma_start(out=mas9[24 * dy:24 * (dy + 1)], in_=apd)
                return mas9

            def stage_B(st):
                """mas conv part 1: PE matmuls + Act silu (exact, via LUT)."""
                m9f = _fl(st['mas9'])
                msl = mpool.tile([G, QF], BF, tag="msl")
                for c2 in range(4):
                    pm = smpool.tile([C, 512], F32, tag="sm")
                    nc.tensor.matmul(pm[0:G, :], cbf_sb[0:72, CB_MAS:CB_MAS + 8],
                                     m9f[:, 512 * c2:512 * (c2 + 1)],
                                     start=True, stop=True)
                    nc.scalar.activation(msl[:, 512 * c2:512 * (c2 + 1)],
                                         pm[0:G, :], AF.Silu,
                                         bias=sc(S_MB1, G), scale=1.0)
                st['msl'] = msl

            def stage_C(st):
                """Contrast stage: DVE-heavy with Pool offload."""
                x, x_odd = st['x'], st['xo']
                t_tiles = []
                for s in (1, 3):
                    nr = GR + s
                    wd = W + s + 1  # even extended width for diag/col dirs
                    Db = dpool.tile([C, 11, XW], BF, tag="d")
                    nc.gpsimd.tensor_tensor(
                        Db[:, 0:nr, 4:4 + W], x[:, J0:J0 + nr, XP:XP + W],
                        x[:, J0 - s:J0 - s + nr, XP:XP + W], ALU.subtract)
                    ub = upool.tile([C, GR, W], BF, tag="u")
                    nc.gpsimd.tensor_tensor(
                        ub[:], Db[:, 0:GR, 4:4 + W], Db[:, s:s + GR, 4:4 + W],
                        ALU.mult)
                    Da = dpool.tile([C, 11, XW], BF, tag="d")
                    nc.vector.tensor_tensor(
                        Da[:, 0:nr, 4:4 + wd], x[:, J0:J0 + nr, XP:XP + wd],
                        x_odd[:, J0 - s:J0 - s + nr, XP - s - 1:XP - s - 1 + wd],
                        ALU.subtract)
                    Doa = dopool.tile([C, GR, W], BF, tag="do")
                    nc.scalar.copy(
                        Doa[:], Da[:, s:s + GR, s + 4:s + 4 + W])
                    ua = upool.tile([C, GR, W], BF, tag="u")
                    nc.vector.tensor_tensor(
                        ua[:], Da[:, 0:GR, 4:4 + W], Doa[:], ALU.mult)
                    m01 = trpool.tile([C, QF], BF, tag="tr")
                    nc.vector.tensor_tensor(m01[:], _fl(ua), _fl(ub), ALU.min)
                    M01 = trpool.tile([C, QF], BF, tag="tr")
                    nc.vector.tensor_tensor(M01[:], _fl(ua), _fl(ub), ALU.max)

                    Dc = dpool.tile([C, 11, XW], BF, tag="d")
                    nc.vector.tensor_tensor(
                        Dc[:, 0:nr, 4:4 + wd],
                        x_odd[:, J0:J0 + nr, XP - s - 1:XP - s - 1 + wd],
                        x[:, J0 - s:J0 - s + nr, XP:XP + wd], ALU.subtract)
                    Doc = dopool.tile([C, GR, W], BF, tag="do")
                    nc.scalar.copy(
                        Doc[:], Dc[:, 0:GR, s + 4:s + 4 + W])
                    uc = upool.tile([C, GR, W], BF, tag="u")
                    nc.vector.tensor_tensor(
                        uc[:], Doc[:], Dc[:, s:s + GR, 4:4 + W], ALU.mult)
                    Dd = dpool.tile([C, 11, XW], BF, tag="d")
                    nc.vector.tensor_tensor(
                        Dd[:, 0:GR, 4:4 + wd], x[:, J0:J0 + GR, XP:XP + wd],
                        x_odd[:, J0:J0 + GR, XP - s - 1:XP - s - 1 + wd],
                        ALU.subtract)
                    Dod = dopool.tile([C, GR, W], BF, tag="do")
                    nc.scalar.copy(
                        Dod[:], Dd[:, 0:GR, s + 4:s + 4 + W])
                    ud = upool.tile([C, GR, W], BF, tag="u")
                    nc.vector.tensor_tensor(
                        ud[:], Dd[:, 0:GR, 4:4 + W], Dod[:], ALU.mult)

                    m23 = trpool.tile([C, QF], BF, tag="tr")
                    nc.vector.tensor_tensor(m23[:], _fl(uc), _fl(ud), ALU.min)
                    min4 = tmppool.tile([C, QF], BF, tag="tmp")
                    nc.vector.tensor_tensor(min4[:], m01[:], m23[:], ALU.min)
                    M23 = trpool.tile([C, QF], BF, tag="tr")
                    nc.vector.tensor_tensor(M23[:], _fl(uc), _fl(ud), ALU.max)
                    max4 = tmppool.tile([C, QF], BF, tag="tmp")
                    nc.vector.tensor_tensor(max4[:], M01[:], M23[:], ALU.max)

                    # t_s = -(w0*max4 + w1m*(ua+ub+uc+ud) + w2*min4): pure
                    # linear combine -> 6 accumulating PE matmuls per chunk
                    # with pre-scaled identity weights, evacuated by Act.
                    t_s = tpool.tile([C, QF], BF, tag="ts")
                    ufl = [_fl(ua), _fl(ub), _fl(uc), _fl(ud)]
                    for c2 in range(4):
                        cc = slice(512 * c2, 512 * (c2 + 1))
                        pt = pxpool.tile([C, 512], F32, tag="px")
                        for i, uf in enumerate(ufl):
                            nc.tensor.matmul(
                                pt[:], cbf_sb[:, CB_EYE:CB_EYE + C],
                                uf[:, cc], start=(i == 0), stop=False)
                        nc.tensor.matmul(pt[:],
                                         cbf_sb[:, CB_EYE + C:CB_EYE + 2 * C],
                                         min4[:, cc], start=False, stop=False)
                        nc.tensor.matmul(pt[:],
                                         cbf_sb[:, CB_EYE + 2 * C:CB_EYE + 3 * C],
                                         max4[:, cc], start=False, stop=True)
                        nc.scalar.activation(t_s[:, cc], pt[:], AF.Identity,
                                             bias=0.0, scale=1.0)
                    t_tiles.append(t_s)

                t1, t3 = t_tiles
                mn = tpool.tile([C, QF], BF, tag="ts")
                nc.vector.tensor_tensor(mn[:], t1[:], t3[:], ALU.min)
                st['t1'], st['t3'], st['mn'] = t1, t3, mn

            def stage_D(mh, st, cen_sb, gate16):
                """bc+bn+silu, w_out, gate combine (Pool), bcast+final, out."""
                base = mh * HB
                t1, t3, mn = st['t1'], st['t3'], st['mn']
                zq = zpool.tile([C, QF], BF, tag="zq")
                for c2 in range(4):
                    cc = slice(512 * c2, 512 * (c2 + 1))
                    pb = pxpool.tile([C, 512], F32, tag="px")
                    nc.tensor.matmul(pb[:], cbf_sb[:, CB_BC:CB_BC + C],
                                     mn[:, cc], start=True, stop=False)
                    nc.tensor.matmul(pb[:], cbf_sb[:, CB_BC + C:CB_BC + 2 * C],
                                     t1[:, cc], start=False, stop=False)
                    nc.tensor.matmul(pb[:], cbf_sb[:, CB_BC + 2 * C:CB_BC + 3 * C],
                                     t3[:, cc], start=False, stop=True)
                    nc.scalar.activation(zq[:, cc], pb[:], AF.Silu,
                                         bias=sc(S_BNB), scale=sc(S_BNS))

                om_q = gpool.tile([G, QF], BF, tag="om")
                for c2 in range(4):
                    pu = smpool.tile([C, 512], F32, tag="sm")
                    nc.tensor.matmul(pu[0:G, :], cbf_sb[:, CB_WOUT:CB_WOUT + 8],
                                     zq[:, 512 * c2:512 * (c2 + 1)],
                                     start=True, stop=True)
                    nc.scalar.activation(om_q[:, 512 * c2:512 * (c2 + 1)],
                                         pu[0:G, :], AF.Tanh,
                                         bias=sc(S_BOUT, G), scale=0.5)

                # h_mm at base partition 0 (HW: SBUF STT operands must share
                # base partition), then copied to gate40 rows 32:40 (weight
                # s1p); row 8 is constant 1.0 (weight s3p); rows 0:8 =
                # (h_mm+s0p/s2p)*h_om (weight s2p).
                mm_q = mpool.tile([G, QF], BF, tag="mm")
                nc.scalar.activation(mm_q[:], st['msl'][:], AF.Tanh,
                                     bias=sc(S_MB2, G), scale=sc(S_MW2, G))
                nc.vector.tensor_copy(gate16[32:40, :], mm_q[:])
                bt = mpool.tile([G, QF], BF, tag="bt")
                nc.scalar.activation(bt[:], mm_q[:], AF.Identity,
                                     bias=sc(S_BT0, G), scale=sc(S_BT1, G))
                nc.vector.tensor_tensor(gate16[0:G, :], om_q[:], bt[:], ALU.mult)

                # final multiply: GPSIMD cannot read PSUM on HW, so half the
                # groups multiply on DVE straight from PSUM (fp32, 1x) and
                # half go Act-evac (bf16) -> Pool multiply.
                for g in range(G):
                    out_t = opool.tile([C, GR, W], BF, tag="o")
                    lh = cbf_sb[0:40, CB_BCAST + g * C:CB_BCAST + (g + 1) * C]
                    for h in range(2):
                        pg = pgpool.tile([C, 1024], F32, tag="pg")
                        for c4 in range(2):
                            c2 = 2 * h + c4
                            nc.tensor.matmul(
                                pg[:, 512 * c4:512 * (c4 + 1)], lh,
                                gate16[:, 512 * c2:512 * (c2 + 1)],
                                start=True, stop=True)
                        r0 = base + 8 * g + 4 * h
                        cs = cen_sb[:, 3 + r0:3 + r0 + 4, :]
                        ot = out_t[:, 4 * h:4 * h + 4, :]
                        pgv = pg[:].rearrange("p (a b) -> p a b", a=4)
                        if g % 2 == 0:
                            nc.vector.tensor_tensor(ot, cs, pgv, ALU.mult)
                        else:
                            gs = trpool.tile([C, QF], BF, tag="tr")
                            gsv = gs[:, 0:1024].rearrange(
                                "p (a b) -> p a b", a=4)
                            nc.scalar.activation(gsv, pgv, AF.Identity,
                                                 bias=0.0, scale=1.0)
                            nc.vector.tensor_tensor(ot, cs, gsv, ALU.mult)
                    eng = nc.scalar if g % 4 == 3 else nc.sync
                    eng.dma_start(
                        out=out_d[:, base + 8 * g:base + 8 * g + GR, :],
                        in_=out_t[:])

            def emit_all():
                # PE p-state warmup: dummy matmuls so the clock ramps while
                # cen streams in (results never read)
                pw = pxpool.tile([C, 512], F32, tag="px")
                for i in range(10):
                    nc.tensor.matmul(pw[:], cbf_sb[:, 0:C],
                                     cbf_sb[:, 0:512], start=(i == 0),
                                     stop=(i == 9))
                cen_sb = cenpool.tile([C, CH + 6, W], BF, tag="cen")
                nrows = [18, 18, 17, 17, 32, 32]
                r = 0
                for i, n in enumerate(nrows):
                    nc.sync.dma_start(out=cen_sb[:, r:r + n, :],
                                      in_=cen_d[:, r:r + n, :])
                    r += n
                cen_fl = _fl(cen_sb)
                gate16 = gpool.tile([40, QF], BF, tag="g16")
                nc.gpsimd.memset(gate16[0:32, :], 1.0)
                # x ring buffers allocated up front so their constant zero
                # col-pads are set once, off the critical path
                xs = []
                for _ in range(MH):
                    xt = xpool.tile([C, XR, XW], BF, tag="x")
                    nc.gpsimd.memset(xt[:, :, 0:XP], 0.0)
                    nc.gpsimd.memset(xt[:, :, W + XP:XW], 0.0)
                    xs.append(xt)
                # software-pipelined over the two macro-halves
                st0 = stage_A(0, cen_fl, xs[0])
                stage_B(st0)
                mas9_1 = emit_mas9(1)
                stage_C(st0)
                st1 = stage_A(1, cen_fl, xs[1], with_mas9=False)
                st1['mas9'] = mas9_1
                stage_D(0, st0, cen_sb, gate16)
                stage_B(st1)
                stage_C(st1)
                stage_D(1, st1, cen_sb, gate16)

            rep_ctx = (tc.For_i(0, loop_reps, 1) if loop_reps
                       else contextlib.nullcontext())
            with rep_ctx:
                emit_all()
    nc.compile()
    return nc


def _softmax(v):
    e = np.exp(v - v.max())
    return e / e.sum()


def _prep_consts(inp):
    w = _softmax(inp['scale1'])
    v = _softmax(inp['scale2'])
    s3 = _softmax(inp['scale3'])
    inv = inp['bn_gamma'] / np.sqrt(inp['bn_var'] + BN_EPS)
    bnb = inp['bn_beta'] - inp['bn_mean'] * inv

    w_in_blk = np.zeros((C, G, C), np.float32)
    for g in range(G):
        w_in_blk[:, g, CR * g:CR * (g + 1)] = inp['w_in'].T
    bc_blk = np.kron(np.eye(G), inp['bc_w'].T)
    A, B = v[0] - v[2], v[1] / 2.0 + v[2]
    wout_lhsT = np.kron(np.eye(G), inp['w_out'][0][:, None]).astype(np.float32)
    k_flat = inp['mas_w1'][0, 0].reshape(9)
    mas_lhsT = np.kron(k_flat[:, None], np.eye(G)).astype(np.float32)
    # gate in tanh basis: om = (1+h_om)/2, mm = (1+h_mm)/2 with
    # h = tanh(pre/2); gate = s3p + s0p*h_om + s1p*h_mm + s2p*h_om*h_mm
    s0p = s3[0] / 2 + s3[2] / 4
    s1p = s3[1] / 2 + s3[2] / 4
    s2p = s3[2] / 4
    s3p = s3[3] + s3[0] / 2 + s3[1] / 2 + s3[2] / 4
    bcast = np.zeros((40, G, C), np.float32)
    for g in range(G):
        bcast[g, g, :] = 1.0
        bcast[8, g, :] = s3p
        bcast[32 + g, g, :] = s1p

    cbf = np.zeros((C, NCB), np.float32)
    cbf[:, CB_WIN:CB_WIN + G * C] = w_in_blk.reshape(C, G * C)
    cbf[:, CB_BC:CB_BC + C] = A * bc_blk
    cbf[:, CB_BC + C:CB_BC + 2 * C] = B * bc_blk
    cbf[:, CB_BC + 2 * C:CB_BC + 3 * C] = B * bc_blk
    cbf[:, CB_WOUT:CB_WOUT + 8] = wout_lhsT
    cbf[0:72, CB_MAS:CB_MAS + 8] = mas_lhsT
    cbf[0:40, CB_BCAST:CB_BCAST + G * C] = bcast.reshape(40, G * C)
    eye = np.eye(C, dtype=np.float32)
    cbf[:, CB_EYE:CB_EYE + C] = (-w[1] / 4.0) * eye
    cbf[:, CB_EYE + C:CB_EYE + 2 * C] = -w[2] * eye
    cbf[:, CB_EYE + 2 * C:CB_EYE + 3 * C] = -w[0] * eye

    b_in_t = np.tile(inp['b_in'], G)[:, None].astype(np.float32)
    cf32 = np.zeros((C, NCF), np.float32)
    cf32[:, S_BIN:S_BIN + 1] = b_in_t
    cf32[:, S_BNS:S_BNS + 1] = np.tile(inv, G)[:, None]
    cf32[:, S_BNB:S_BNB + 1] = np.tile(bnb, G)[:, None]
    scal = [(S_NW0, -w[0]), (S_NW1_4, -w[1] / 4.0), (S_NW2, -w[2]),
            (S_BT0, s0p), (S_BT1, s2p),
            (S_BOUT, inp['b_out'][0] / 2.0), (S_MB1, inp['mas_b1'][0]),
            (S_MW2, inp['mas_w2'][0, 0] / 2.0),
            (S_MB2, inp['mas_b2'][0] / 2.0)]
    for col, val in scal:
        cf32[:, col] = val
    return {'cbf': cbf.astype(ml_dtypes.bfloat16), 'cf32': cf32,
            'b_in_t': b_in_t}


def make_in_maps(inp, consts):
    in_maps = []
    for core in range(N_CORES):
        b, hf = core // 2, core % 2
        r0 = CH * hf
        cen_pad = np.pad(inp['cen'][b], ((0, 0), (3, 3), (0, 0)))
        cen_core = np.ascontiguousarray(cen_pad[:, r0:r0 + CH + 6, :])
        mas_pad = np.pad(inp['mas'][b, 0], ((1, 1), (1, 1)))
        mas_core = np.ascontiguousarray(mas_pad[r0:r0 + CH + 2, :])
        cf32 = consts['cf32'].copy()
        if hf != 0:
            cf32[:, S_BTOP:S_BTOP + 1] = consts['b_in_t']
        if hf != 1:
            cf32[:, S_BBOT:S_BBOT + 1] = consts['b_in_t']
        in_maps.append({
            'cen_bf': cen_core.astype(ml_dtypes.bfloat16),
            'mas': mas_core.astype(ml_dtypes.bfloat16),
            'cbf': consts['cbf'],
            'cf32': cf32,
        })
    return in_maps


def run(inputs, trace=False):
    inp = {k: np.asarray(v) for k, v in inputs.items()}
    consts = _prep_consts(inp)
    if 'nc' not in _CACHE:
        _CACHE['nc'] = build_nc()
    nc = _CACHE['nc']
    in_maps = make_in_maps(inp, consts)
    res = run_bass_kernel_spmd(nc, in_maps, list(range(N_CORES)), trace=trace)
    out = np.empty((4, C, H, W), np.float32)
    for core in range(N_CORES):
        b, hf = core // 2, core % 2
        out[b, :, CH * hf:CH * (hf + 1), :] = \
            res.results[core]['out'].astype(np.float32)
    return out, res


def kernel(**inputs):
    return run(inputs)[0]


def bench(inputs, iters=30, reps=0):
    """Time repeated executions with device-resident inputs (no donation)."""
    import time
    import jax
    from jax.sharding import Mesh, PartitionSpec
    from jax.experimental.shard_map import shard_map
    from concourse import bass2jax

    inp = {k: np.asarray(v) for k, v in inputs.items()}
    consts = _prep_consts(inp)
    key = ('nc', reps)
    if key not in _CACHE:
        _CACHE[key] = build_nc(loop_reps=reps)
    nc = _CACHE[key]
    in_maps = make_in_maps(inp, consts)

    bass2jax.install_neuronx_cc_hook()
    in_names, out_names, out_avals, zero_outs = [], [], [], []
    pname = (nc.partition_id_tensor.name if nc.partition_id_tensor else None)
    for alloc in nc.m.functions[0].allocations:
        if not isinstance(alloc, mybir.MemoryLocationSet):
            continue
        name = alloc.memorylocations[0].name
        if alloc.kind == "ExternalInput":
            if name != pname:
                in_names.append(name)
        elif alloc.kind == "ExternalOutput":
            out_names.append(name)
            out_avals.append(jax.core.ShapedArray(
                tuple(alloc.tensor_shape), mybir.dt.np(alloc.dtype)))
            zero_outs.append(np.zeros(tuple(alloc.tensor_shape),
                                      mybir.dt.np(alloc.dtype)))
    n_params = len(in_names)

    def _body(*args):
        operands = list(args)
        all_names = in_names + out_names
        if nc.partition_id_tensor:
            operands.append(bass2jax.partition_id_tensor())
            all_names = all_names + [nc.partition_id_tensor.name]
        outs = bass2jax._bass_exec_p.bind(
            *operands, out_avals=tuple(out_avals), in_names=tuple(all_names),
            out_names=tuple(out_names), lowering_input_output_aliases=(),
            sim_require_finite=True, sim_require_nnan=True, nc=nc)
        return tuple(outs)

    devices = jax.devices()[:N_CORES]
    mesh = Mesh(np.asarray(devices), ("core",))
    nin = n_params + len(out_names)
    sharded = jax.jit(
        shard_map(_body, mesh=mesh,
                  in_specs=(PartitionSpec("core"),) * nin,
                  out_specs=(PartitionSpec("core"),) * len(out_names),
                  check_rep=False),
        donate_argnums=tuple(range(n_params, nin)),
        keep_unused=True,
    )
    concat_in = [np.concatenate([in_maps[c][nm] for c in range(N_CORES)], 0)
                 for nm in in_names]
    concat_zero = [np.zeros((N_CORES * z.shape[0], *z.shape[1:]), z.dtype)
                   for z in zero_outs]
    sh = jax.sharding.NamedSharding(mesh, PartitionSpec("core"))
    dev_in = [jax.device_put(a, sh) for a in concat_in]
    prev = jax.device_put(concat_zero[0], sh)

    outs = sharded(*dev_in, prev)
    jax.block_until_ready(outs)
    result = np.asarray(outs[0]).copy()
    prev = outs[0]
    times = []
    for _ in range(iters):
        t0 = time.perf_counter()
        outs = sharded(*dev_in, prev)
        jax.block_until_ready(outs)
        times.append(time.perf_counter() - t0)
        prev = outs[0]

    full = np.empty((4, C, H, W), np.float32)
    arr = result.reshape(N_CORES, C, CH, W)
    for core in range(N_CORES):
        b, hf = core // 2, core % 2
        full[b, :, CH * hf:CH * (hf + 1), :] = arr[core].astype(np.float32)
    return full, times
